# revision 30
# baseline (speedup 1.0000x reference)
"""Trainium2 Bass kernel for the BINN convnet problem — transposed pipeline (v6).

Computation (per row b of inp, all column indices mod D=128):
    x[b, j]  = (c1[j] * a[b, j+1] - c2[j] * a[b, j-2]) * a[b, j-1]
    out      = x + a @ W_lin.T + b_lin
with c1[j] = w[j,0]*w[j,2], c2[j] = w[j,1]*w[j,2], except j==1 where the
outer factor is w[1,0] instead of w[1,2].

v6 strategy: compute in TRANSPOSED (feature-on-partition) space, in a
rotated output basis o'[p, b] = out[b, (p+1) mod D]:

  x'[p, b] = g'[p, b] * aT[p, b]        (partition-aligned! no shifts)
  g'       = RotG @ aT   (RotG[p,:] = G[p+1,:], constant STATIONARY)
  mm'      = RotW @ aT   (RotW[p,:] = W_lin[p+1,:])
  o'       = x' + mm' + bias'[p]        (bias is PER-PARTITION here)

Per 512-row subtile:
  1. CAST_NUM/CAST_MOD of subtiles: GpSimd casts the natural fp32 tile to
     fp16 (SBUF->SBUF) and PE transposes at 1 cycle/row; the rest transpose
     the fp32 tile directly (2 cycles/row) — this splits the cast load
     between the slow GpSimd and the PE, balancing all five engines.
  2. PE transposes per 128-block into PSUM.
  3. ScalarE evacuates aT (PSUM->SBUF, always fp16 — converts on the fp32 path).
  4. PE: P = RotG16 @ aT (start=True, stop=False), moving N=512 fp16.
  5. DVE: P *= aT in place (PSUM read-modify-write).
  6. PE: P += RotW16 @ aT (start=False, stop=True) -- PSUM accumulation
     performs the x + mm add for free.
  7. ScalarE/DVE split: o' = P + bias' (per-partition bias), fp16 to SBUF.
  8. Store o' to a transposed DRAM output [D, nrows]; the host undoes the
     (transpose, 16-row interleave, +1 column rotation) while upcasting.

Memory: reads 33.5 MB fp32, writes 16.8 MB fp16 per core -> ~145 us DMA
roofline at the measured ~355 GB/s per-core DMA rate.
"""

import os
import sys

import numpy as np

if os.path.isdir("/opt/trn_rl_repo") and "/opt/trn_rl_repo" not in sys.path:
    sys.path.insert(0, "/opt/trn_rl_repo")

import concourse.mybir as mybir
import concourse.tile as tile
from concourse import bacc
from concourse.bass_utils import run_bass_kernel_spmd

D = 128          # feature dim
N_CORES = 8
SUB = 512        # rows per compute subtile
DMA_ROWS = 2048  # rows per DMA tile
QB = SUB // D    # 128-row blocks per subtile (4)
F32 = mybir.dt.float32
F16 = mybir.dt.float16
EVAC_ACT = 296   # columns of the aT evacuation handled by ScalarE (rest on DVE)
CAST_MOD = 9     # of every 9 subtiles, CAST_NUM (evenly spread) go through the
CAST_NUM = 5     # GpSimd fp16-cast + fp16-transpose path; the rest are
                 # transposed directly from fp32 (PE pays 2 cyc/row there).


def build_program(nrows: int):
    assert nrows % DMA_ROWS == 0
    ndma = nrows // DMA_ROWS
    nsub = DMA_ROWS // SUB
    nsubs = ndma * nsub

    nc = bacc.Bacc("TRN2", debug=False, target_bir_lowering=False)

    inp = nc.declare_dram_parameter("inp", [nrows, D], F32, isOutput=False)
    gT = nc.declare_dram_parameter("gT", [D, D], F16, isOutput=False)
    wT = nc.declare_dram_parameter("wT", [D, D], F16, isOutput=False)
    bias = nc.declare_dram_parameter("bias", [D, 1], F32, isOutput=False)
    ident = nc.declare_dram_parameter("ident", [D, D], F16, isOutput=False)
    ident32 = nc.declare_dram_parameter("ident32", [D, D], F32, isOutput=False)
    outT = nc.declare_dram_parameter("outT", [D, nrows], F16, isOutput=True)

    with tile.TileContext(nc) as tc:
        with (
            tc.tile_pool(name="const", bufs=1) as const_pool,
            tc.tile_pool(name="a_sb", bufs=4) as a_pool,
            tc.tile_pool(name="ab_sb", bufs=8) as ab_pool,
            tc.tile_pool(name="at_sb", bufs=8) as at_pool,
            tc.tile_pool(name="o_sb", bufs=4) as o_pool,
            tc.tile_pool(name="at_ps", bufs=3, space="PSUM") as atps_pool,
            tc.tile_pool(name="p_ps", bufs=5, space="PSUM") as p_pool,
        ):
            gT_sb = const_pool.tile([D, D], F16)
            wT_sb = const_pool.tile([D, D], F16)
            bias_sb = const_pool.tile([D, 1], F32)
            id_sb = const_pool.tile([D, D], F16)
            id32_sb = const_pool.tile([D, D], F32)
            nc.sync.dma_start(out=gT_sb[:], in_=gT[:, :])
            nc.sync.dma_start(out=wT_sb[:], in_=wT[:, :])
            nc.sync.dma_start(out=bias_sb[:], in_=bias[:, :])
            nc.sync.dma_start(out=id_sb[:], in_=ident[:, :])
            nc.sync.dma_start(out=id32_sb[:], in_=ident32[:, :])

            st = {}  # k -> dict of live tiles

            def emit_conv(k):
                td, ts = divmod(k, nsub)
                if ts == 0:
                    r0 = td * DMA_ROWS
                    a_sb = a_pool.tile([D, DMA_ROWS], F32, tag="a")
                    src = inp[r0 : r0 + DMA_ROWS, :].rearrange(
                        "(p q) d -> p q d", p=D
                    )
                    nc.sync.dma_start(
                        out=a_sb[:].rearrange("p (q d) -> p q d", d=D), in_=src
                    )
                    o_sb = o_pool.tile([D, DMA_ROWS], F16, tag="o")
                else:
                    prev = st[k - 1]
                    a_sb, o_sb = prev["a"], prev["o"]
                f0 = ts * SUB
                # spread cast subtiles evenly (avoid bursts on GpSimd)
                if k >= 8 and (k * CAST_NUM) % CAST_MOD < CAST_NUM:
                    ab = ab_pool.tile([D, SUB], F16, tag="ab")
                    nc.gpsimd.tensor_copy(out=ab[:], in_=a_sb[:, f0 : f0 + SUB])
                else:
                    ab = None  # fp32 transpose path reads a_sb directly
                st[k] = {"td": td, "f0": f0, "a": a_sb, "o": o_sb, "ab": ab}

            def emit_trans(k):
                s = st[k]
                ab = s["ab"]
                if ab is not None:
                    src, idm, dt, lo = ab, id_sb, F16, 0
                else:
                    src, idm, dt, lo = s["a"], id32_sb, F32, s["f0"]
                at_ps = atps_pool.tile([D, SUB], dt, tag="atps")
                for q in range(QB):
                    nc.tensor.matmul(
                        out=at_ps[:, q * D : (q + 1) * D],
                        lhsT=src[:, lo + q * D : lo + (q + 1) * D],
                        rhs=idm[:],
                        is_transpose=True,
                        start=True,
                        stop=True,
                    )
                s["atps"] = at_ps

            def emit_evac(k):
                s = st[k]
                at = at_pool.tile([D, SUB], F16, tag="at")
                nc.scalar.copy(
                    out=at[:, 0:EVAC_ACT], in_=s["atps"][:, 0:EVAC_ACT]
                )
                nc.vector.tensor_copy(
                    out=at[:, EVAC_ACT:SUB], in_=s["atps"][:, EVAC_ACT:SUB]
                )
                s["at"] = at

            def emit_gmm(k):
                s = st[k]
                P = p_pool.tile([D, SUB], F32, tag="p")
                nc.tensor.matmul(
                    out=P[:], lhsT=gT_sb[:], rhs=s["at"][:], start=True, stop=False
                )
                s["P"] = P

            def emit_mul(k):
                s = st[k]
                nc.vector.tensor_mul(out=s["P"][:], in0=s["P"][:], in1=s["at"][:])

            def emit_wmm(k):
                s = st[k]
                nc.tensor.matmul(
                    out=s["P"][:], lhsT=wT_sb[:], rhs=s["at"][:], start=False, stop=True
                )

            def emit_bcopy(k):
                s = st[k]
                P, o_sb, f0 = s["P"], s["o"], s["f0"]
                nc.scalar.add(
                    out=o_sb[:, f0 : f0 + ACT_COLS],
                    in_=P[:, 0:ACT_COLS],
                    add=bias_sb[:, 0:1],
                )
                nc.vector.tensor_scalar_add(
                    out=o_sb[:, f0 + ACT_COLS : f0 + SUB],
                    in0=P[:, ACT_COLS:SUB],
                    scalar1=bias_sb[:, 0:1],
                )

            def emit_store(k):
                td, ts = divmod(k, nsub)
                if ts == nsub - 1:
                    c0 = td * DMA_ROWS
                    nc.scalar.dma_start(
                        out=outT[:, c0 : c0 + DMA_ROWS], in_=st[k]["o"][:]
                    )

            # 8-stage pipeline; per engine, oldest work is emitted first.
            for step in range(nsubs + 7):
                if step >= 7 and step - 7 < nsubs:
                    emit_store(step - 7)
                if step >= 6 and step - 6 < nsubs:
                    emit_bcopy(step - 6)
                if step >= 5 and step - 5 < nsubs:
                    emit_wmm(step - 5)
                if step >= 4 and step - 4 < nsubs:
                    emit_mul(step - 4)
                if step >= 3 and step - 3 < nsubs:
                    emit_gmm(step - 3)
                if step >= 2 and step - 2 < nsubs:
                    emit_evac(step - 2)
                if step >= 1 and step - 1 < nsubs:
                    emit_trans(step - 1)
                if step < nsubs:
                    emit_conv(step)

    nc.compile()
    return nc


def make_consts(w: np.ndarray, W_lin: np.ndarray, b_lin: np.ndarray):
    w = np.asarray(w, np.float64)
    c1 = w[:, 0] * w[:, 2]
    c2 = w[:, 1] * w[:, 2]
    c1[1] = w[1, 0] * w[1, 0]
    c2[1] = w[1, 1] * w[1, 0]

    j = np.arange(D)
    G = np.zeros((D, D), np.float64)
    G[j, (j + 1) % D] += c1
    G[j, (j - 2) % D] -= c2

    rot = (j + 1) % D  # output partition p holds natural column p+1
    RotG = G[rot, :]
    RotW = np.asarray(W_lin, np.float64)[rot, :]
    gT = np.ascontiguousarray(RotG.T).astype(np.float16)
    wT = np.ascontiguousarray(RotW.T).astype(np.float16)
    bias = np.asarray(b_lin, np.float32)[rot].reshape(D, 1)
    ident = np.eye(D, dtype=np.float16)
    ident32 = np.eye(D, dtype=np.float32)
    return {"gT": gT, "wT": wT, "bias": bias, "ident": ident, "ident32": ident32}


_PROGRAM_CACHE: dict[int, object] = {}
TRACE = False
TRACE_DIR = None
LAST_RESULT = None


def _get_program(nrows: int):
    if nrows not in _PROGRAM_CACHE:
        _PROGRAM_CACHE[nrows] = build_program(nrows)
    return _PROGRAM_CACHE[nrows]


def _unscramble(shard_t: np.ndarray, nrows: int) -> np.ndarray:
    """[D, nrows] fp16 device output -> [nrows, D] natural-layout fp16.

    Device column c = td*2048 + s16*128 + n holds row td*2048 + n*16 + s16;
    device partition p holds natural output column (p+1) mod D.
    """
    T = nrows // DMA_ROWS
    V = shard_t.reshape(D, T, DMA_ROWS // D, D)  # [p, td, s16, n]
    U = V.transpose(1, 3, 2, 0).reshape(nrows, D)  # [row, p]
    return np.roll(U, 1, axis=1)


def kernel(**inputs) -> np.ndarray:
    inp = np.ascontiguousarray(np.asarray(inputs["inp"], np.float32))
    w = np.asarray(inputs["w"], np.float32)
    W_lin = np.asarray(inputs["W_lin"], np.float32)
    b_lin = np.asarray(inputs["b_lin"], np.float32)

    B = inp.shape[0]
    assert inp.shape[1] == D and B % N_CORES == 0
    nrows = B // N_CORES

    consts = make_consts(w, W_lin, b_lin)
    shards = inp.reshape(N_CORES, nrows, D)

    nc = _get_program(nrows)
    in_maps = [{"inp": shards[i], **consts} for i in range(N_CORES)]
    res = run_bass_kernel_spmd(
        nc, in_maps, list(range(N_CORES)), trace=TRACE, tmpdir=TRACE_DIR
    )
    global LAST_RESULT
    LAST_RESULT = res

    out = np.empty((B, D), np.float32)
    for i in range(N_CORES):
        shard_t = np.asarray(res.results[i]["outT"])
        out[i * nrows : (i + 1) * nrows] = _unscramble(shard_t, nrows)
    return out


if __name__ == "__main__":
    rng = np.random.default_rng(0)
    B = N_CORES * DMA_ROWS * 2
    inp = rng.standard_normal((B, D)).astype(np.float32)
    w = rng.random((D, 3)).astype(np.float32)
    W_lin = (rng.standard_normal((D, D)) / np.sqrt(D)).astype(np.float32)
    b_lin = (rng.standard_normal(D) * 0.01).astype(np.float32)
    dt = np.ones(1, np.float32)

    actual = kernel(inp=inp, dt=dt, w=w, W_lin=W_lin, b_lin=b_lin)

    a = inp.astype(np.float64)
    c1 = (w[:, 0] * w[:, 2]).astype(np.float64)
    c2 = (w[:, 1] * w[:, 2]).astype(np.float64)
    c1[1] = w[1, 0] * w[1, 0]
    c2[1] = w[1, 1] * w[1, 0]
    ap1 = np.roll(a, -1, 1)
    am2 = np.roll(a, 2, 1)
    am1 = np.roll(a, 1, 1)
    x = (c1 * ap1 - c2 * am2) * am1
    expected = x + a @ W_lin.astype(np.float64).T + b_lin
    err = np.abs(actual - expected).max() / np.abs(expected).max()
    print("scale-relative absmax err:", err)
